# revision 3
# baseline (speedup 1.0000x reference)
"""Trainium2 Bass kernel for 2-layer GraphConv + linear head (GCN-style).

Distribution: nodes (and incident edges by destination) are partitioned
across 8 NeuronCores; weights replicated; per-layer node-feature tables
exchanged with 4 chunked AllGathers per layer so aggregation of chunk c
overlaps the AllGather of later chunks.

Performance architecture (~4.8x over the fp32 direct port):
  * bf16 tables/weights/PE operands, fp32 accumulation (tolerance 2e-2).
  * No element-granule (AP-rearrange) DMAs: features are host-pre-permuted
    and transposed; every device DMA moves >=256B contiguous runs.
  * The per-edge row fetch is one dma_gather per (dst tile, src chunk)
    (~2.2k tokens each). SWDGE descriptor generation on the GpSimd Q7 costs
    ~7.5ns/token and is the critical path (~97% busy), so the token count
    is minimized: 2 fixed base slots per (dst, chunk) plus dst-sorted
    overflow windows (window counts are max-over-cores for SPMD).
  * Scatter-add on the PE: layer 1 computes out[dst, feat] (one-hot
    stationary; per-node norm is a per-partition scalar; table rows are
    written contiguously), layer 2 computes out[feat, dst] (gathered window
    stationary), feeding the w2/w3 matmuls without a transpose.
  * Overflow one-hots are 256 columns wide, rebased at each window's lowest
    populated 128-dst group, and built in one broadcast is_equal per
    (tile, chunk), prefetched 3 steps ahead so the DVE FIFO never stalls
    the gather pipeline.
  * Chunk-major accumulation into persistent fp32 SBUF slabs; finalize is
    broadcast tensor_tensor ops + ACT relu; g-table writes batched 4 tiles
    at a time, split only at chunk boundaries; AllGathers of g trigger as
    soon as their rows are written.
"""

import numpy as np
import ml_dtypes

import concourse.bass as bass
import concourse.bacc as bacc
import concourse.tile as tile
import concourse.mybir as mybir
from concourse import bass_utils

F32 = mybir.dt.float32
BF16 = mybir.dt.bfloat16
I16 = mybir.dt.int16
NPBF16 = ml_dtypes.bfloat16

NC_CORES = 8
NCHUNK = 4
SLOTS = 2
WIN = 128
WPD = WIN // SLOTS  # 64 dsts per base window
DGRP = 128          # dst partition-group width (layer-1 matmul M)


def cdiv(a, b):
    return -(-a // b)


class Plan:
    def __init__(self, n_nodes, e_subgraph, tile_d=512):
        N = n_nodes
        assert N % NC_CORES == 0
        self.N = N
        NLOC = self.NLOC = N // NC_CORES
        # chunk = contiguous local-row range; one zero row per chunk
        real_pc = self.real_pc = cdiv(NLOC, NCHUNK)
        L = self.L = real_pc + 1
        assert NC_CORES * L <= 32767, (L, NC_CORES * L)
        self.TABROWS = NC_CORES * L
        self.CONTRIB = NCHUNK * L
        self.CPAD = cdiv(self.CONTRIB, 128) * 128   # featsP rows (padded)
        self.NBLK = self.CPAD // 128
        self.ZROW = L - 1                           # zero row (rank 0 region)
        self.TILE_D = tile_d
        self.NT = cdiv(NLOC, tile_d)
        self.PADLOC = self.NT * tile_d
        self.NG = cdiv(tile_d, DGRP)

        src = np.asarray(e_subgraph[0], dtype=np.int64)
        dst = np.asarray(e_subgraph[1], dtype=np.int64)

        deg = np.bincount(src, minlength=N).astype(np.float32)
        self.norm = np.clip(deg, 1.0, None) ** -0.5

        # src -> (chunk, row within chunk table)
        s_owner = src // NLOC
        s_loc = src % NLOC
        s_cc = np.minimum(s_loc // real_pc, NCHUNK - 1)
        s_row = s_owner * L + (s_loc - s_cc * real_pc)
        d_owner = dst // NLOC
        d_loc = dst % NLOC

        # per-tile real dst count and base window count
        self.D_t = [min(tile_d, NLOC - t * tile_d) for t in range(self.NT)]
        self.nbase = [cdiv(d * SLOTS, WIN) for d in self.D_t]

        # per-core edge grouping
        per_core = []
        for c in range(NC_CORES):
            sel = d_owner == c
            dl, cc, row = d_loc[sel], s_cc[sel], s_row[sel]
            order = np.lexsort((cc, dl))
            dl, cc, row = dl[order], cc[order], row[order]
            key = dl * NCHUNK + cc
            if len(key):
                is_new = np.r_[True, key[1:] != key[:-1]]
                grp_start = np.flatnonzero(is_new)
                rank = np.arange(len(key)) - grp_start[np.cumsum(is_new) - 1]
            else:
                rank = key
            per_core.append((dl, cc, row, rank))

        # base slot arrays + overflow lists (tile-wide, dst-sorted)
        NT, TD = self.NT, tile_d
        base = [np.full((NT, NCHUNK, self.nbase[0] * WIN), self.ZROW, np.int64)
                for _ in range(NC_CORES)]
        ovf = [[[([], []) for _ in range(NCHUNK)] for _ in range(NT)]
               for _ in range(NC_CORES)]
        for c in range(NC_CORES):
            dl, cc, row, rank = per_core[c]
            t = dl // TD
            din = dl - t * TD
            bm = rank < SLOTS
            base[c][t[bm], cc[bm], din[bm] * SLOTS + rank[bm]] = row[bm]
            om = ~bm
            for tt, ci, dd, rr in zip(t[om], cc[om], din[om], row[om]):
                ovf[c][tt][ci][0].append(rr)
                ovf[c][tt][ci][1].append(dd)

        # static overflow window counts (max over cores -> SPMD uniform)
        self.nw = np.zeros((NT, NCHUNK), np.int64)
        for t in range(NT):
            for ci in range(NCHUNK):
                mx = max(len(ovf[c][t][ci][0]) for c in range(NC_CORES))
                self.nw[t, ci] = cdiv(mx, WIN) if mx else 0

        # tokens per (t, cc)
        self.T = np.zeros((NT, NCHUNK), np.int64)
        for t in range(NT):
            for ci in range(NCHUNK):
                self.T[t, ci] = (self.nbase[t] + int(self.nw[t, ci])) * WIN
        self.nwin = self.T // WIN
        self.nwin_max = int(self.nwin.max())
        self.nw_tc_max = int(self.nw.max())
        assert int(self.T.max()) // 16 + 1 <= 480, int(self.T.max())
        # idx columns ordered cc-major: for cc: for t
        self.cols_cc = [int(self.T[:, ci].sum()) // 16 for ci in range(NCHUNK)]
        self.cols_cc_max = max(self.cols_cc)
        self.totcols = sum(self.cols_cc)
        self.nw_tot = int(self.nw.sum())

        # build per-core idx / dstloc; track per-window dgrp span (union of
        # cores) so layer 1 emits matmuls only for populated dgrps
        self.idx = np.zeros((NC_CORES, 128, self.totcols), np.int16)
        self.dstloc = np.full((NC_CORES, 128, max(self.nw_tot, 1)), -1.0,
                              np.float32)
        self.wgrp = [[[set() for _ in range(int(self.nw[t, ci]))]
                      for ci in range(NCHUNK)] for t in range(NT)]
        for c in range(NC_CORES):
            col = 0
            w_i = 0
            for ci in range(NCHUNK):
                for t in range(NT):
                    toks = np.full(int(self.T[t, ci]), self.ZROW, np.int64)
                    nb = self.nbase[t] * WIN
                    toks[:nb] = base[c][t, ci][:nb]
                    rr, dd = ovf[c][t][ci]
                    if len(rr):
                        toks[nb:nb + len(rr)] = rr
                    for j in range(int(self.nw[t, ci])):
                        sl = dd[j * WIN:(j + 1) * WIN]
                        if len(sl):
                            self.dstloc[c, :len(sl), w_i] = sl
                            for g in set(d // DGRP for d in sl):
                                self.wgrp[t][ci][j].add(int(g))
                        w_i += 1
                    seg = int(self.T[t, ci])
                    wrapped = toks.astype(np.int16).reshape(seg // 16, 16).T
                    self.idx[c, :, col:col + seg // 16] = np.tile(wrapped, (8, 1))
                    col += seg // 16
            assert col == self.totcols
            assert w_i == self.nw_tot
        # rebase each window's dstloc at its lowest populated dgrp; one-hot
        # width is then 2*DGRP (spans are <= 2 dgrps by construction check)
        self.gbase = [[[min(g) if g else 0 for g in self.wgrp[t][ci]]
                       for ci in range(NCHUNK)] for t in range(NT)]
        w_i = 0
        for ci in range(NCHUNK):
            for t in range(NT):
                for j in range(int(self.nw[t, ci])):
                    gb = self.gbase[t][ci][j]
                    assert all(g - gb in (0, 1) for g in self.wgrp[t][ci][j])
                    if gb:
                        m = self.dstloc[:, :, w_i] >= 0
                        self.dstloc[:, :, w_i][m] -= gb * DGRP
                    w_i += 1
        # dstloc column offset for (cc, t): cc-major order
        self.woff = np.zeros((NT, NCHUNK), np.int64)
        w_i = 0
        for ci in range(NCHUNK):
            for t in range(NT):
                self.woff[t, ci] = w_i
                w_i += int(self.nw[t, ci])

        # node -> table/y row mapping: row = l + chunk(l)
        self.row_of_node = (np.arange(NLOC) +
                            np.minimum(np.arange(NLOC) // real_pc, NCHUNK - 1))
        # AG-g trigger: batch-last tile after which chunk cg's rows are all
        # written (batches of 4 tiles; row = node + chunk(node))
        self.ag_trigger = {}
        for cg in range(NCHUNK):
            need_node = min((cg + 1) * real_pc, NLOC) - 1  # last real node
            b = need_node // (4 * tile_d)
            t_last = min(b * 4 + 3, self.NT - 1)
            self.ag_trigger.setdefault(t_last, []).append(cg)
        assert sum(len(v) for v in self.ag_trigger.values()) == NCHUNK

    # ---- host-side tensors -------------------------------------------------
    def feats_pre(self, features_core):
        """[din, CPAD] bf16, permuted so featsP[:, r] = x[node(r)] (or 0)."""
        din = features_core.shape[1]
        fp = np.zeros((self.CPAD, din), np.float32)
        fp[self.row_of_node] = features_core
        return np.ascontiguousarray(fp.T).astype(NPBF16)

    def norm_arrays(self, core):
        nl = self.norm[core * self.NLOC:(core + 1) * self.NLOC]
        # per y-row norm, [128, NBLK] (column b = rows 128b..128b+128)
        npr = np.ones(self.CPAD, np.float32)
        npr[self.row_of_node] = nl
        normP = np.ascontiguousarray(npr.reshape(self.NBLK, 128).T)
        # per dst node, partition-major per dgrp: [128, NT*NG]
        nd = np.ones(self.NT * self.TILE_D, np.float32)
        nd[:self.NLOC] = nl
        normcol = np.ascontiguousarray(
            nd.reshape(self.NT * self.NG, DGRP).T)
        # per dst node broadcast over partitions: [128, PADLOC]
        normb = np.broadcast_to(nd, (128, self.PADLOC)).copy()
        return normP, normcol, normb

    def g_write_segments(self):
        """Batched g-table writes. Returns list of (tile_lo, n_tiles, pieces).
        pieces: ("mid", j0, nj, r0)  = SBUF [:, j0:j0+nj, :] -> rows r0..+nj*128
                ("part", p0, pn, j, r0) = SBUF [p0:p0+pn, j, :] -> rows r0..+pn
        Batches of up to 4 tiles, split at chunk boundaries & clamped to NLOC."""
        segs = []
        BT = 4
        for b0 in range(0, self.NT, BT):
            bt = min(BT, self.NT - b0)
            n0 = b0 * self.TILE_D
            n1 = min(n0 + bt * self.TILE_D, self.NLOC)
            cuts = [n0, n1]
            for k in range(1, NCHUNK):
                cb = k * self.real_pc
                if n0 < cb < n1:
                    cuts.append(cb)
            cuts = sorted(set(cuts))
            pieces = []
            for a, b in zip(cuts[:-1], cuts[1:]):
                row = int(a + min(a // self.real_pc, NCHUNK - 1))
                d = a - n0          # batch-local dst index
                end = b - n0
                while d < end:
                    if d % 128 or end - d < 128:
                        pn = min(128 - d % 128, end - d)
                        pieces.append(("part", d % 128, pn, d // 128, row))
                        d += pn
                        row += pn
                    else:
                        nj = (end - d) // 128
                        pieces.append(("mid", d // 128, nj, row))
                        d += nj * 128
                        row += nj * 128
            segs.append((b0, bt, pieces))
        return segs

    def consts(self):
        # onesb2[j]: [128, 128] bf16, token tok -> dst 64j + tok//SLOTS
        onesb2 = np.zeros((2, WIN, 128), np.float32)
        for j in range(2):
            for tk in range(WIN):
                onesb2[j, tk, WPD * j + tk // SLOTS] = 1.0
        # onesb64: [128, 64] for layer-2 moving operand
        onesb64 = np.zeros((WIN, WPD), np.float32)
        for tk in range(WIN):
            onesb64[tk, tk // SLOTS] = 1.0
        iota256 = np.broadcast_to(
            np.arange(2 * DGRP, dtype=np.float32),
            (128, 1, 2 * DGRP)).copy()
        return (onesb2.astype(NPBF16), onesb64.astype(NPBF16), iota256)


def build_nc(plan: Plan, din, dh, dout):
    p = plan
    nc = bacc.Bacc("TRN2", target_bir_lowering=False, debug=False,
                   num_devices=NC_CORES)
    TD, NG, NT = p.TILE_D, p.NG, p.NT
    kt = din // 128
    MAXNW = p.nw_tc_max

    featsP_d = nc.dram_tensor("featsP", [din, p.CPAD], BF16, kind="ExternalInput")
    w1_d = nc.dram_tensor("w1", [din, dh], BF16, kind="ExternalInput")
    w2_d = nc.dram_tensor("w2", [dh, dh], BF16, kind="ExternalInput")
    w3t_d = nc.dram_tensor("w3t", [dh, dout], BF16, kind="ExternalInput")
    b1x4_d = nc.dram_tensor("b1x4", [128, NG * dh], F32, kind="ExternalInput")
    b2_d = nc.dram_tensor("b2", [dh, 1], F32, kind="ExternalInput")
    b3_d = nc.dram_tensor("b3", [dout, 1], F32, kind="ExternalInput")
    normP_d = nc.dram_tensor("normP", [128, p.NBLK], F32, kind="ExternalInput")
    normcol_d = nc.dram_tensor("normcol", [128, NT * NG], F32,
                               kind="ExternalInput")
    normb_d = nc.dram_tensor("normb", [128, p.PADLOC], F32, kind="ExternalInput")
    idx_d = nc.dram_tensor("idx", [128, p.totcols], I16, kind="ExternalInput")
    dstloc_d = nc.dram_tensor("dstloc", [128, max(p.nw_tot, 1)], F32,
                              kind="ExternalInput")
    onesb2_d = nc.dram_tensor("onesb2", [128, 2, 128], BF16, kind="ExternalInput")
    onesb64_d = nc.dram_tensor("onesb64", [128, WPD], BF16, kind="ExternalInput")
    iota256_d = nc.dram_tensor("iota256", [128, 1, 2 * DGRP], F32,
                               kind="ExternalInput")
    out_d = nc.dram_tensor("outT", [dout, p.PADLOC], F32, kind="ExternalOutput")

    y_loc = nc.dram_tensor("y_loc", [p.CPAD, dh], BF16)
    g_loc = nc.dram_tensor("g_loc", [p.CPAD, dh], BF16)
    t_y = [nc.dram_tensor(f"t_y{c}", [p.TABROWS, dh], BF16, addr_space="Shared")
           for c in range(NCHUNK)]
    t_g = [nc.dram_tensor(f"t_g{c}", [p.TABROWS, dh], BF16, addr_space="Shared")
           for c in range(NCHUNK)]

    rg = [list(range(NC_CORES))]
    SUPB = 8   # phase-A node blocks per supertile (write batch)
    PSB = 4    # phase-A blocks per psum group

    with tile.TileContext(nc) as tc:
        with (
            tc.tile_pool(name="const", bufs=1) as cp,
            tc.tile_pool(name="xt", bufs=2) as xtp,
            tc.tile_pool(name="yb", bufs=2) as ybp,
            tc.tile_pool(name="gath", bufs=2) as gp,
            tc.tile_pool(name="oh", bufs=4) as ohp,
            tc.tile_pool(name="mid", bufs=2) as midp,
            tc.tile_pool(name="gw", bufs=2) as gwp,
            tc.tile_pool(name="psA", bufs=2, space="PSUM") as psAp,
            tc.tile_pool(name="psAg", bufs=3, space="PSUM") as psGp,
            tc.tile_pool(name="ps2", bufs=1, space="PSUM") as ps2p,
            tc.tile_pool(name="ps3", bufs=2, space="PSUM") as ps3p,
        ):
            # ---- constants / residents ----
            w1_sb = cp.tile([128, kt, dh], BF16)
            for k in range(kt):
                nc.sync.dma_start(w1_sb[:, k, :], w1_d[k * 128:(k + 1) * 128, :])
            w2_sb = cp.tile([128, dh], BF16)
            nc.sync.dma_start(w2_sb[:], w2_d[:, :])
            w3t_sb = cp.tile([128, dout], BF16)
            nc.sync.dma_start(w3t_sb[:], w3t_d[:, :])
            b1x4_sb = cp.tile([128, NG * dh], F32)
            nc.sync.dma_start(b1x4_sb[:], b1x4_d[:, :])
            b2_sb = cp.tile([dh, 1], F32)
            nc.sync.dma_start(b2_sb[:], b2_d[:, :])
            b3_sb = cp.tile([dout, 1], F32)
            nc.sync.dma_start(b3_sb[:], b3_d[:, :])
            normP_sb = cp.tile([128, p.NBLK], F32)
            nc.sync.dma_start(normP_sb[:], normP_d[:, :])
            normcol_sb = cp.tile([128, NT * NG], F32)
            nc.sync.dma_start(normcol_sb[:], normcol_d[:, :])
            dstloc_sb = cp.tile([128, max(p.nw_tot, 1)], F32)
            nc.sync.dma_start(dstloc_sb[:], dstloc_d[:, :])
            onesb2_sb = cp.tile([128, 2, 128], BF16)
            nc.sync.dma_start(onesb2_sb[:], onesb2_d[:, :, :])
            onesb64_sb = cp.tile([128, WPD], BF16)
            nc.sync.dma_start(onesb64_sb[:], onesb64_d[:, :])
            iota256_sb = cp.tile([128, 1, 2 * DGRP], F32)
            nc.sync.dma_start(iota256_sb[:], iota256_d[:, :, :])
            ix_sb = cp.tile([128, p.totcols], I16)
            nc.sync.dma_start(ix_sb[:], idx_d[:, :])
            zeros_sb = cp.tile([NCHUNK, dh], BF16)
            nc.vector.memset(zeros_sb[:], 0.0)
            # persistent fp32 accumulators, one [128, TD] slab per dst tile
            accS = cp.tile([128, NT, TD], F32)

            # g-table zero rows (one per chunk: row cc*L + ZROW)
            nc.sync.dma_start(
                g_loc[p.ZROW:p.ZROW + 1 + (NCHUNK - 1) * p.L:p.L, :],
                zeros_sb[:, :])

            # ---- phase A: y = (x * norm) @ w1, node-major ----
            for st0 in range(0, p.NBLK, SUPB):
                nb = min(SUPB, p.NBLK - st0)
                r0 = st0 * 128
                xt = xtp.tile([128, kt, SUPB * 128], BF16, tag="xt")
                for k in range(kt):
                    nc.sync.dma_start(
                        xt[:, k, :nb * 128],
                        featsP_d[k * 128:(k + 1) * 128, r0:r0 + nb * 128])
                yb = ybp.tile([128, SUPB, dh], BF16, tag="yb")
                for j0 in range(0, nb, PSB):
                    jn = min(PSB, nb - j0)
                    ps = psAp.tile([128, PSB, dh], F32, space="PSUM", tag="psA")
                    for j in range(jn):
                        for k in range(kt):
                            nc.tensor.matmul(
                                ps[:, j, :],
                                xt[:, k, (j0 + j) * 128:(j0 + j + 1) * 128],
                                w1_sb[:, k, :],
                                start=(k == 0), stop=(k == kt - 1))
                    nc.vector.tensor_tensor(
                        out=yb[:, j0:j0 + jn, :], in0=ps[:, :jn, :],
                        in1=normP_sb[:, st0 + j0:st0 + j0 + jn]
                        .broadcast_to([128, jn, dh]),
                        op=mybir.AluOpType.mult)
                nc.sync.dma_start(
                    y_loc[r0:r0 + nb * 128, :]
                    .rearrange("(j q) f -> q j f", q=128),
                    yb[:, :nb, :])

            # ---- AllGather y chunks ----
            for ci in range(NCHUNK):
                nc.gpsimd.collective_compute(
                    "AllGather", mybir.AluOpType.bypass,
                    ins=[y_loc[ci * p.L:(ci + 1) * p.L, :]],
                    outs=[t_y[ci].ap()],
                    replica_groups=rg)

            # ---- aggregation layer (shared emission) ----
            def build_oh(t, ci):
                """Batched one-hot build for all ovf windows of (t, ci);
                depends only on constants, so it can run far ahead."""
                nw_tc = int(p.nw[t, ci])
                if not nw_tc:
                    return None
                oh = ohp.tile([128, max(MAXNW, 1), 2 * DGRP], BF16, tag="oh",
                              name="ohT")
                w0 = int(p.woff[t, ci])
                nc.vector.tensor_tensor(
                    out=oh[:, :nw_tc, :],
                    in0=iota256_sb[:, :, :]
                    .broadcast_to([128, nw_tc, 2 * DGRP]),
                    in1=dstloc_sb[:, w0:w0 + nw_tc]
                    .broadcast_to([128, nw_tc, 2 * DGRP]),
                    op=mybir.AluOpType.is_equal)
                return oh

            def tile_windows(t, ci, g_t, woff_in_pair, layer, oh):
                """Emit PE scatter + fold for one (t, ci)."""
                ps = psGp.tile([128, TD], F32, space="PSUM", tag="psAg",
                               name="psAgT")
                nw_tc = int(p.nw[t, ci])
                wgrps = [sorted(g) or [0] for g in p.wgrp[t][ci]]
                if layer == 1:
                    total_mm = p.nbase[t] + sum(len(g) for g in wgrps)
                else:
                    total_mm = p.nbase[t] + nw_tc
                mi = 0
                wi = 0
                for w in range(p.nbase[t]):
                    gap = g_t[:, woff_in_pair + wi, :]
                    first, last = mi == 0, mi == total_mm - 1
                    if layer == 1:
                        nc.tensor.matmul(
                            ps[:, (w // 2) * 128:(w // 2 + 1) * 128],
                            onesb2_sb[:, w % 2, :], gap,
                            start=first, stop=last)
                    else:
                        nc.tensor.matmul(
                            ps[:, w * WPD:(w + 1) * WPD],
                            gap, onesb64_sb[:],
                            start=first, stop=last)
                    wi += 1
                    mi += 1
                for j in range(nw_tc):
                    gap = g_t[:, woff_in_pair + wi, :]
                    gb = p.gbase[t][ci][j]
                    if layer == 1:
                        for g in wgrps[j]:
                            first, last = mi == 0, mi == total_mm - 1
                            k = g - gb
                            nc.tensor.matmul(
                                ps[:, g * DGRP:(g + 1) * DGRP],
                                oh[:, j, k * DGRP:(k + 1) * DGRP], gap,
                                start=first, stop=last)
                            mi += 1
                    else:
                        first, last = mi == 0, mi == total_mm - 1
                        w2 = min(2 * DGRP, TD - gb * DGRP)
                        nc.tensor.matmul(
                            ps[:, gb * DGRP:gb * DGRP + w2], gap,
                            oh[:, j, :w2],
                            start=first, stop=last)
                        mi += 1
                    wi += 1
                # fold into fp32 SBUF accumulator
                if ci == 0:
                    nc.vector.tensor_copy(accS[:, t, :], ps[:])
                else:
                    nc.vector.tensor_tensor(
                        out=accS[:, t, :], in0=accS[:, t, :], in1=ps[:],
                        op=mybir.AluOpType.add)

            def agg_layer(tables, layer):
                gseg = p.g_write_segments() if layer == 1 else None
                gwt = {}
                seq = [(t, ci) for ci in range(NCHUNK) for t in range(NT)]
                ohs = {}
                PF = 3
                for k0 in range(PF):
                    ohs[seq[k0]] = build_oh(*seq[k0])
                cols = {}
                col = 0
                for ci in range(NCHUNK):
                    for t in range(NT):
                        cols[(t, ci)] = col
                        col += int(p.T[t, ci]) // 16
                for k, (t, ci) in enumerate(seq):
                    if k + PF < len(seq):
                        ohs[seq[k + PF]] = build_oh(*seq[k + PF])
                    Tt = int(p.T[t, ci])
                    nwin_t = Tt // WIN
                    g_t = gp.tile([128, p.nwin_max, dh], BF16, tag="g",
                                  name="gtile")
                    col = cols[(t, ci)]
                    nc.gpsimd.dma_gather(
                        g_t[:, :nwin_t, :], tables[ci][:, :],
                        ix_sb[:, col:col + Tt // 16], Tt, Tt, dh,
                        single_packet=False)
                    tile_windows(t, ci, g_t, 0, layer, ohs.pop((t, ci)))
                    if True:
                        if ci == NCHUNK - 1:
                            if layer == 1:
                                fin1(t, gwt, gseg)
                                for cg in p.ag_trigger.get(t, []):
                                    nc.gpsimd.collective_compute(
                                        "AllGather", mybir.AluOpType.bypass,
                                        ins=[g_loc[cg * p.L:(cg + 1) * p.L, :]],
                                        outs=[t_g[cg].ap()],
                                        replica_groups=rg)
                            else:
                                fin2(t)

            def fin1(t, gwt, gseg):
                """H1 = relu(acc*norm + b1); G = H1*norm -> g rows (batched)."""
                a3 = accS[:, t, :].rearrange("q (g d) -> q g d", g=NG)
                ncol = normcol_sb[:, t * NG:(t + 1) * NG] \
                    .broadcast_to([128, NG, DGRP])
                tmp = midp.tile([128, NG, DGRP], F32, tag="fa")
                nc.vector.tensor_tensor(out=tmp[:], in0=a3, in1=ncol,
                                        op=mybir.AluOpType.mult)
                tmp2 = midp.tile([128, NG * DGRP], F32, tag="fb")
                nc.vector.tensor_tensor(
                    out=tmp2[:], in0=tmp[:].rearrange("q g d -> q (g d)"),
                    in1=b1x4_sb[:, :TD], op=mybir.AluOpType.add)
                tmp3 = midp.tile([128, NG, DGRP], F32, tag="fa")
                nc.scalar.activation(
                    tmp3[:].rearrange("q g d -> q (g d)"), tmp2[:],
                    mybir.ActivationFunctionType.Relu)
                b0 = (t // 4) * 4
                if b0 not in gwt:
                    gwt[b0] = gwp.tile([128, 4 * NG, dh], BF16, tag="gw",
                                       name="gb")
                gb = gwt[b0]
                nc.vector.tensor_tensor(
                    out=gb[:, (t - b0) * NG:(t - b0 + 1) * NG, :],
                    in0=tmp3[:], in1=ncol, op=mybir.AluOpType.mult)
                # emit batched writes when the batch's last tile is done
                for (tb0, bt, pieces) in gseg:
                    if tb0 == b0 and t == b0 + bt - 1:
                        for pc in pieces:
                            if pc[0] == "mid":
                                _, j0, nj, r0 = pc
                                nc.sync.dma_start(
                                    g_loc[r0:r0 + nj * 128, :]
                                    .rearrange("(j q) f -> q j f", q=128),
                                    gb[:, j0:j0 + nj, :])
                            else:
                                _, p0, pn, j, r0 = pc
                                nc.sync.dma_start(
                                    g_loc[r0:r0 + pn, :],
                                    gb[p0:p0 + pn, j, :])

            def fin2(t):
                """out = relu((acc@w2)*norm + b2) @ w3t + b3."""
                d0 = t * TD
                a2 = midp.tile([128, TD], BF16, tag="fc")
                nc.vector.tensor_copy(a2[:], accS[:, t, :])
                nb = midp.tile([128, TD], F32, tag="fe")
                nc.sync.dma_start(nb[:], normb_d[:, d0:d0 + TD])
                ps2 = ps2p.tile([128, TD], F32, space="PSUM", tag="ps2")
                nc.tensor.matmul(ps2[:], w2_sb[:], a2[:], start=True, stop=True)
                h = midp.tile([128, TD], F32, tag="fa")
                nc.vector.tensor_tensor(out=h[:], in0=ps2[:], in1=nb[:],
                                        op=mybir.AluOpType.mult)
                h2 = midp.tile([128, TD], BF16, tag="fc")
                nc.scalar.activation(h2[:], h[:],
                                     mybir.ActivationFunctionType.Relu,
                                     bias=b2_sb[:, 0:1])
                ps3 = ps3p.tile([dout, TD], F32, space="PSUM", tag="ps3")
                nc.tensor.matmul(ps3[:], w3t_sb[:], h2[:], start=True, stop=True)
                ot = midp.tile([dout, TD], F32, tag="fd")
                nc.scalar.activation(ot[:], ps3[:],
                                     mybir.ActivationFunctionType.Identity,
                                     bias=b3_sb[:, 0:1])
                nc.sync.dma_start(out_d[:, d0:d0 + TD], ot[:])

            agg_layer(t_y, layer=1)
            agg_layer(t_g, layer=2)

    nc.compile()
    return nc


def make_in_maps(plan: Plan, features, w1, b1, w2, b2, w3, b3):
    p = plan
    onesb2, onesb64, iota256 = p.consts()
    NG = p.NG
    dh = w1.shape[1]
    b1x4 = np.broadcast_to(np.tile(np.asarray(b1, np.float32), NG),
                           (128, NG * dh)).copy()
    in_maps = []
    for c in range(NC_CORES):
        normP, normcol, normb = p.norm_arrays(c)
        in_maps.append(dict(
            featsP=p.feats_pre(np.asarray(
                features[c * p.NLOC:(c + 1) * p.NLOC], np.float32)),
            w1=np.ascontiguousarray(w1, np.float32).astype(NPBF16),
            w2=np.ascontiguousarray(w2, np.float32).astype(NPBF16),
            w3t=np.ascontiguousarray(np.asarray(w3).T, np.float32).astype(NPBF16),
            b1x4=b1x4,
            b2=np.asarray(b2, np.float32).reshape(-1, 1),
            b3=np.asarray(b3, np.float32).reshape(-1, 1),
            normP=normP, normcol=normcol, normb=normb,
            idx=p.idx[c], dstloc=p.dstloc[c],
            onesb2=np.ascontiguousarray(onesb2.transpose(1, 0, 2)),
            onesb64=onesb64, iota256=iota256,
        ))
    return in_maps


def assemble_output(plan: Plan, results, dout):
    p = plan
    h = np.empty((p.N, dout), np.float32)
    for c in range(NC_CORES):
        h[c * p.NLOC:(c + 1) * p.NLOC] = results[c]["outT"][:, :p.NLOC].T
    return h


def run_graphconv(n_nodes, e_subgraph, features, w1, b1, w2, b2, w3, b3,
                  tile_d=512, mode="hw", trace=False):
    plan = Plan(n_nodes, e_subgraph, tile_d=tile_d)
    nc = build_nc(plan, features.shape[1], w1.shape[1], w3.shape[0])
    in_maps = make_in_maps(plan, features, w1, b1, w2, b2, w3, b3)
    if mode == "sim":
        from concourse import bass_interp
        sim = bass_interp.MultiCoreSim(nc, num_cores=NC_CORES)
        for c in range(NC_CORES):
            for k, v in in_maps[c].items():
                sim.cores[c].tensor(k)[:] = v
        sim.simulate(check_with_hw=False)
        results = [{"outT": sim.cores[c].mem_tensor("outT")}
                   for c in range(NC_CORES)]
        res = None
    else:
        res = bass_utils.run_bass_kernel_spmd(
            nc, in_maps, list(range(NC_CORES)), trace=trace)
        results = res.results
    h = assemble_output(plan, results, w3.shape[0])
    return h, res


def kernel(n_subgraph, e_subgraph, to_fetch, features, w1, b1, w2, b2, w3, b3):
    h, _ = run_graphconv(
        n_subgraph.shape[0], e_subgraph, features, w1, b1, w2, b2, w3, b3)
    return (h, h)
